# revision 1
# baseline (speedup 1.0000x reference)
"""Trainium2 Bass kernel for DepthBBoxProcessor.

For each of 4096 bboxes: 7x7 bilinear grid-sample on the depth map of the
box's image, mean over the 49 samples, appended as column 7 of the output.

Key observations exploited:
  * The 7x7 sample grid has ~1px spacing, so every box's 49 samples (and
    their bilinear corners) live inside one 8x8 pixel patch of its image.
  * Only the MEAN of the samples is needed and bilinear weights are
    separable, so  mean = (1/49) * WY^T @ Patch @ WX  where WX/WY are
    accumulated per-column / per-row weight vectors. No per-sample gather.
  * HW indirect DMA gathers one contiguous window per partition per call.
    The depth map is repacked on host into 8-row x 4-col pixel blocks so a
    16-row x 12-col "supertile" covering any box's patch is reachable with
    just TWO 96-float windows (three horizontally-adjacent blocks each).
  * Schedule: the gather-index chain is emitted first so the 10 indirect
    DMAs (the Pool-engine bottleneck) start ASAP; the bin-weight math runs
    on the vector engine underneath the gathers; the tail is a fused
    multiply+accumulate per group.

Sharding (8 cores): depth_map is sharded 2 images/core by batch dim; boxes
are routed on host to the core holding their image (batch-id-aware
routing), padded to a fixed 640/core. Host unpermutes per-core results and
concatenates with the (unchanged) input boxes.
"""

import os
import sys

import numpy as np

if "/opt/trn_rl_repo" not in sys.path:
    sys.path.insert(0, "/opt/trn_rl_repo")

import concourse.bacc as bacc
import concourse.bass as bass
import concourse.mybir as mybir
import concourse.tile as tile
from concourse.tile import add_dep_helper
from concourse.bass_utils import run_bass_kernel_spmd

H, W = 1080, 1920
HW = H * W
B = 16
N_CORES = 8
IMGS_PER_CORE = B // N_CORES
S = 640          # padded boxes per core (actual max ~550)
G = S // 128     # free-dim groups of 128 boxes
NBX4 = W // 4    # 480 block cols (x blocks are 4 px wide)
NBY = H // 8     # 135 block rows (y blocks are 8 px tall)
F32 = mybir.dt.float32
I32 = mybir.dt.int32
ALU = mybir.AluOpType
AX = mybir.AxisListType
ACTF = mybir.ActivationFunctionType

# const layout (one row, replicated to 128 partitions):
# [0:7]   off_x - 1    (grid x offsets, center fold of "-1.0")
# [7:14]  off_y - 1
# [14:30] iota 1..16   (j+1 values for the cumulative-clamp weights)
# [30:32] [0, 32*NBX4] window starts for the two vertical block rows
_C_OFFX, _C_OFFY, _C_IOTA, _C_W2 = 0, 7, 14, 30
_C_TOT = 32


def _const_row() -> np.ndarray:
    f = np.float32
    offx = np.linspace(-3.0, 3.0, 7).astype(f) / f(W * 0.5) - f(1.0)
    offy = np.linspace(-3.0, 3.0, 7).astype(f) / f(H * 0.5) - f(1.0)
    iota = np.arange(16, dtype=f) + f(1.0)
    w2 = np.array([0.0, 32 * NBX4], dtype=f)
    row = np.concatenate([offx, offy, iota, w2])
    return np.tile(row[None, :], (128, 1)).astype(f)


def _axis_origin(nc, pool, bb, cst, c0, c1, scale, clampmax, blk, nblk, span, tag):
    """Box coords -> (pix [128,G,7] sample px coords, b0 [128,G,1]
    supertile block origin on this axis)."""
    v = nc.vector
    ctr = pool.tile([128, G, 1], F32, tag=f"ctr{tag}")
    v.tensor_tensor(out=ctr[:], in0=bb[:, :, c0:c0 + 1], in1=bb[:, :, c1:c1 + 1], op=ALU.add)
    s = pool.tile([128, G, 7], F32, tag=f"s{tag}")
    off = cst[:, _C_OFFX + (0 if tag == "x" else 7):][:, :7]
    v.tensor_tensor(out=s[:], in0=ctr[:].to_broadcast([128, G, 7]),
                    in1=off.unsqueeze(1).to_broadcast([128, G, 7]), op=ALU.add)
    g = pool.tile([128, G, 7], F32, tag=f"g{tag}")
    v.tensor_scalar(out=g[:], in0=s[:], scalar1=1.0, scalar2=-1.0, op0=ALU.min, op1=ALU.max)
    pix = pool.tile([128, G, 7], F32, tag=f"pix{tag}")
    v.tensor_scalar(out=pix[:], in0=g[:], scalar1=scale, scalar2=scale, op0=ALU.mult, op1=ALU.add)
    # floor of the CENTER pixel only (HW f32->i32 cast rounds to nearest-even)
    ri = pool.tile([128, G, 1], I32, tag=f"ri{tag}")
    v.tensor_copy(out=ri[:], in_=pix[:, :, 3:4])
    rf = pool.tile([128, G, 1], F32, tag=f"rf{tag}")
    v.tensor_copy(out=rf[:], in_=ri[:])
    m = pool.tile([128, G, 1], F32, tag=f"m{tag}")
    v.tensor_tensor(out=m[:], in0=rf[:], in1=pix[:, :, 3:4], op=ALU.is_gt)
    p0c = pool.tile([128, G, 1], F32, tag=f"p0c{tag}")
    v.tensor_tensor(out=p0c[:], in0=rf[:], in1=m[:], op=ALU.subtract)
    # supertile origin: b0 = clamp(floor((clamp(p0c,3,dim-5)-3)/8), ., nblk-2)
    cp3 = pool.tile([128, G, 1], F32, tag=f"cp3{tag}")
    v.tensor_scalar(out=cp3[:], in0=p0c[:], scalar1=3.0, scalar2=clampmax,
                    op0=ALU.max, op1=ALU.min)
    u = pool.tile([128, G, 1], F32, tag=f"u{tag}")
    v.tensor_scalar(out=u[:], in0=cp3[:], scalar1=1.0 / blk, scalar2=3.0 / blk,
                    op0=ALU.mult, op1=ALU.subtract)
    ui = pool.tile([128, G, 1], I32, tag=f"ui{tag}")
    v.tensor_copy(out=ui[:], in_=u[:])
    uf = pool.tile([128, G, 1], F32, tag=f"uf{tag}")
    v.tensor_copy(out=uf[:], in_=ui[:])
    m2 = pool.tile([128, G, 1], F32, tag=f"m2{tag}")
    v.tensor_tensor(out=m2[:], in0=uf[:], in1=u[:], op=ALU.is_gt)
    b0r = pool.tile([128, G, 1], F32, tag=f"b0r{tag}")
    v.tensor_tensor(out=b0r[:], in0=uf[:], in1=m2[:], op=ALU.subtract)
    b0 = pool.tile([128, G, 1], F32, tag=f"b0{tag}")
    v.tensor_scalar(out=b0[:], in0=b0r[:], scalar1=float(nblk - span), scalar2=None,
                    op0=ALU.min)
    return pix, b0


def _axis_bins(nc, pool, cst, pix, b0, blk, nbins, tag, after=None):
    """(pix, b0) -> Wt16 [128,G,16]: accumulated separable bilinear weights
    over the 16 supertile rows/cols (relative to pixel 8*b0).

    Uses the cumulative-clamp identity: a sample at real position t deposits
    (1-frac) at floor(t) and frac at floor(t)+1, so its cumulative weight up
    to bin j is clamp(j+1-t, 0, 1). Then Wt[j] = C[j] - C[j-1]."""
    v = nc.vector
    t = pool.tile([128, G, 7], F32, tag=f"t{tag}")
    t_inst = v.scalar_tensor_tensor(out=t[:], in0=b0[:].to_broadcast([128, G, 7]),
                                    scalar=-float(blk), in1=pix[:], op0=ALU.mult, op1=ALU.add)
    if after is not None:
        # ordering-only edge: keep the gather-index chain ahead of the
        # (longer) weight math on the in-order vector engine
        add_dep_helper(t_inst.ins, after.ins, sync=False,
                       reason="bins yield to gather-index chain")
    iota = cst[:, _C_IOTA:_C_IOTA + nbins]
    a = pool.tile([128, G, nbins, 7], F32, tag=f"a{tag}")
    v.tensor_tensor(out=a[:],
                    in0=iota.unsqueeze(1).unsqueeze(3).to_broadcast([128, G, nbins, 7]),
                    in1=t[:].unsqueeze(2).to_broadcast([128, G, nbins, 7]),
                    op=ALU.subtract)
    cl = pool.tile([128, G, nbins, 7], F32, tag=f"cl{tag}")
    v.tensor_scalar(out=cl[:], in0=a[:], scalar1=1.0, scalar2=0.0, op0=ALU.min, op1=ALU.max)
    cum = pool.tile([128, G, nbins + 1], F32, tag=f"cum{tag}")
    nc.gpsimd.memset(cum[:, :, 0:1], 0.0)
    v.tensor_reduce(out=cum[:, :, 1:nbins + 1], in_=cl[:], axis=AX.X, op=ALU.add)
    wt = pool.tile([128, G, nbins], F32, tag=f"wt{tag}")
    v.tensor_tensor(out=wt[:], in0=cum[:, :, 1:nbins + 1], in1=cum[:, :, 0:nbins],
                    op=ALU.subtract)
    return wt


BLOB_W = G * 7 + _C_TOT   # host-transposed boxes + const row, one DMA


def build_nc(n_iters: int = 1) -> bass.Bass:
    nc = bacc.Bacc()
    # [p, g*7+c] = boxes[g*128+p, c]; [p, G*7:] = per-partition const row
    blob = nc.dram_tensor("blob", [128, BLOB_W], F32, kind="ExternalInput")
    # block-repacked depth: [(img*NBY + by)*NBX4 + bx] * 32 + r*4 + c
    depth = nc.dram_tensor("depth", [IMGS_PER_CORE * HW, 1], F32, kind="ExternalInput")
    avg_out = nc.dram_tensor("avg", [S, 1], F32, kind="ExternalOutput")

    with tile.TileContext(nc) as tc:
        with tc.tile_pool(name="p", bufs=1) as pool:
          for _it in range(n_iters):
              v = nc.vector
              blob_sb = pool.tile([128, BLOB_W], F32, tag="blob")
              nc.sync.dma_start(out=blob_sb[:], in_=blob[:, :])
              bb = blob_sb[:, 0:G * 7].rearrange("p (g c) -> p g c", g=G)
              cst = blob_sb[:, G * 7:BLOB_W]

              # ---- gather-index chain first: unblock the Pool engine ASAP
              pixx, bx0 = _axis_origin(nc, pool, bb, cst, 3, 5, 959.5, float(W - 5), 4, NBX4, 3, "x")
              pixy, by0 = _axis_origin(nc, pool, bb, cst, 4, 6, 539.5, float(H - 5), 8, NBY, 2, "y")

              # window-0 start: img*HW + by0*(32*NBX4) + bx0*32 ; window-1: +32*NBX4.
              # Re-associated so the x-side partial (preX) computes while the
              # later-finishing y-origin chain is still in flight.
              base = pool.tile([128, G, 1], F32, tag="base")
              nc.scalar.activation(out=base[:], in_=bb[:, :, 0:1], func=ACTF.Copy,
                                   scale=float(HW), bias=0.0)
              prex = pool.tile([128, G, 1], F32, tag="prex")
              v.scalar_tensor_tensor(out=prex[:], in0=bx0[:], scalar=32.0, in1=base[:],
                                     op0=ALU.mult, op1=ALU.add)
              st0 = pool.tile([128, G, 1], F32, tag="st0")
              v.scalar_tensor_tensor(out=st0[:], in0=by0[:], scalar=float(32 * NBX4),
                                     in1=prex[:], op0=ALU.mult, op1=ALU.add)
              idxf = pool.tile([128, G, 2], F32, tag="idxf")
              v.tensor_tensor(out=idxf[:], in0=st0[:].to_broadcast([128, G, 2]),
                              in1=cst[:, _C_W2:_C_W2 + 2].unsqueeze(1).to_broadcast([128, G, 2]),
                              op=ALU.add)
              idx = pool.tile([128, G, 2], I32, tag="idx")
              idx_inst = v.tensor_copy(out=idx[:], in_=idxf[:])

              # supertile [g][w][bx][r][c]: 2 windows x 3 blocks x 8 rows x 4 cols.
              # Windows are read 128 floats wide (512B DMA descriptors); only the
              # first 96 floats (3 blocks) are consumed, the tail is in-bounds
              # slack from the next block column.
              st = pool.tile([128, G, 2, 128], F32, tag="st")
              for gi in range(G):
                  for wdw in range(2):
                      nc.gpsimd.indirect_dma_start(
                          out=st[:, gi, wdw, :],
                          out_offset=None,
                          in_=depth[:, :],
                          in_offset=bass.IndirectOffsetOnAxis(
                              ap=idx[:, gi, wdw:wdw + 1], axis=0),
                      )

              # ---- bin weights (runs on DVE underneath the gathers)
              wx = _axis_bins(nc, pool, cst, pixx, bx0, 4, 12, "x", after=idx_inst)
              wy = _axis_bins(nc, pool, cst, pixy, by0, 8, 16, "y", after=idx_inst)

              # weight product in supertile memory order
              wprod = pool.tile([128, G, 192], F32, tag="wprod")
              for gi in range(G):
                  for wdw in range(2):
                      v.tensor_tensor(
                          out=wprod[:, gi, wdw * 96:(wdw + 1) * 96]
                              .rearrange("p (bx r c) -> p bx r c", bx=3, r=8),
                          in0=wy[:, gi, wdw * 8:(wdw + 1) * 8]
                              .unsqueeze(1).unsqueeze(3).to_broadcast([128, 3, 8, 4]),
                          in1=wx[:, gi, :].rearrange("p (bx c) -> p bx c", bx=3)
                              .unsqueeze(2).to_broadcast([128, 3, 8, 4]),
                          op=ALU.mult)

              # fused multiply + accumulate per group: sm[g] = sum(st_g * wprod_g)
              sm = pool.tile([128, G], F32, tag="sm")
              scratch = pool.tile([128, G, 2, 96], F32, tag="scratch")
              for gi in range(G):
                  v.scalar_tensor_tensor(out=scratch[:, gi, :, :], in0=st[:, gi, :, 0:96],
                                         scalar=1.0,
                                         in1=wprod[:, gi, :].rearrange("p (w e) -> p w e", w=2),
                                         op0=ALU.mult, op1=ALU.mult,
                                         accum_out=sm[:, gi:gi + 1])
              av = pool.tile([128, G], F32, tag="av")
              v.tensor_scalar(out=av[:], in0=sm[:], scalar1=float(1.0 / 49.0), scalar2=None,
                              op0=ALU.mult)
              # partition-major store (contiguous per partition: 128 descriptors
              # instead of 640); host transposes back
              nc.sync.dma_start(out=avg_out.rearrange("(p g) o -> p g o", p=128),
                                in_=av[:].unsqueeze(2))
    nc.finalize()
    return nc


_NC_CACHE = None


def _get_nc() -> bass.Bass:
    global _NC_CACHE
    if _NC_CACHE is None:
        _NC_CACHE = build_nc()
    return _NC_CACHE


def _host_bilinear(bb: np.ndarray, dm: np.ndarray) -> np.ndarray:
    """Reference-exact fallback for overflow boxes (host, numpy)."""
    f = np.float32
    bids = bb[:, 0].astype(np.int32)
    cx = bb[:, 3] + bb[:, 5] - f(1.0)
    cy = bb[:, 4] + bb[:, 6] - f(1.0)
    offx = np.linspace(-3.0, 3.0, 7).astype(f) / f(W * 0.5)
    offy = np.linspace(-3.0, 3.0, 7).astype(f) / f(H * 0.5)
    gx = np.clip(cx[:, None] + offx[None, :], -1.0, 1.0).astype(f)
    gy = np.clip(cy[:, None] + offy[None, :], -1.0, 1.0).astype(f)
    ix = ((gx + f(1.0)) * f(0.5) * f(W - 1)).astype(f)
    iy = ((gy + f(1.0)) * f(0.5) * f(H - 1)).astype(f)
    x0 = np.floor(ix); y0 = np.floor(iy)
    wx = ix - x0; wy = iy - y0
    x0i = np.clip(x0.astype(np.int32), 0, W - 1); x1i = np.clip(x0i + 1, 0, W - 1)
    y0i = np.clip(y0.astype(np.int32), 0, H - 1); y1i = np.clip(y0i + 1, 0, H - 1)
    d = dm[:, 0]
    bI = bids[:, None, None]
    vv = (d[bI, y0i[:, :, None], x0i[:, None, :]] * (1 - wx)[:, None, :] * (1 - wy)[:, :, None]
          + d[bI, y0i[:, :, None], x1i[:, None, :]] * wx[:, None, :] * (1 - wy)[:, :, None]
          + d[bI, y1i[:, :, None], x0i[:, None, :]] * (1 - wx)[:, None, :] * wy[:, :, None]
          + d[bI, y1i[:, :, None], x1i[:, None, :]] * wx[:, None, :] * wy[:, :, None])
    return vv.mean(axis=(1, 2)).astype(f)


def run(inputs: dict, trace: bool = False):
    """Returns (full_output [N,8] f32, BassKernelResults)."""
    bb = np.ascontiguousarray(np.asarray(inputs["bboxes"], dtype=np.float32))
    dm = np.ascontiguousarray(np.asarray(inputs["depth_map"], dtype=np.float32))
    n = bb.shape[0]
    bids = bb[:, 0].astype(np.int32)
    core = np.clip(bids // IMGS_PER_CORE, 0, N_CORES - 1)
    sels, overflow = [], []
    for c in range(N_CORES):
        sel = np.nonzero(core == c)[0]
        if len(sel) > S:
            overflow.append(sel[S:])
            sel = sel[:S]
        sels.append(sel)
    in_maps = []
    for c in range(N_CORES):
        sel = sels[c]
        boxes_c = np.zeros((S, 7), np.float32)
        boxes_c[:len(sel)] = bb[sel]
        boxes_c[:len(sel), 0] = np.clip(bids[sel] - IMGS_PER_CORE * c, 0,
                                        IMGS_PER_CORE - 1).astype(np.float32)
        blob = np.concatenate(
            [boxes_c.reshape(G, 128, 7).transpose(1, 0, 2).reshape(128, G * 7),
             _const_row()], axis=1).astype(np.float32)
        d2 = dm[IMGS_PER_CORE * c:IMGS_PER_CORE * (c + 1), 0]
        dblk = np.ascontiguousarray(
            d2.reshape(IMGS_PER_CORE, NBY, 8, NBX4, 4).transpose(0, 1, 3, 2, 4)
        ).reshape(IMGS_PER_CORE * HW, 1)
        in_maps.append({"blob": blob, "depth": dblk})

    nc = _get_nc()
    if os.environ.get("BASS_KERNEL_SIM") == "1":
        from concourse.bass_interp import CoreSim
        res, br = [], None
        for c in range(N_CORES):
            sim = CoreSim(nc)
            for k_, v_ in in_maps[c].items():
                sim.tensor(k_)[:] = v_
            sim.simulate()
            res.append({"avg": np.array(sim.tensor("avg"))})
    else:
        br = run_bass_kernel_spmd(nc, in_maps, list(range(N_CORES)), trace=trace)
        res = br.results

    avg = np.empty((n, 1), np.float32)
    for c in range(N_CORES):
        # device layout is [p, g] flattened p-major; host order is g*128+p
        av_c = res[c]["avg"].reshape(128, G).T.reshape(-1)
        avg[sels[c], 0] = av_c[:len(sels[c])]
    for sel in overflow:
        avg[sel, 0] = _host_bilinear(bb[sel], dm)
    return np.concatenate([bb, avg], axis=1), br


def kernel(**inputs) -> np.ndarray:
    out, _ = run(inputs)
    return out



# revision 22
# speedup vs baseline: 1.0190x; 1.0190x over previous
"""Trainium2 Bass kernel for DepthBBoxProcessor (v3).

For each of 4096 bboxes: 7x7 bilinear grid-sample on the depth map of the
box's image, mean over the 49 samples, appended as column 7 of the output.

Key observations exploited:
  * For this input distribution the grid NEVER clips at the image border
    (|cx| <= 0.9532 < 1), and the 7 sample positions per axis are evenly
    spaced with step s = (dim-1)/dim ~ 0.9995 px.  Approximating s = 1
    (anchored at the CENTER sample; measured max rel err 7e-4 vs the
    reference, gate is 2e-2) collapses the separable accumulated weights
    to a closed form:  wt = [1-f, 1, 1, 1, 1, 1, 1, f].
  * Every box needs only an 8x8 pixel patch.  HW indirect DMA gathers ONE
    contiguous window per partition per instruction (multi-index offset
    APs do not work on silicon), so the depth map is repacked on host into
    8-row BANDS, column-major within the band:
        band[img][y0][c][r] = img[y0 + r][c]
    Then the 8x8 patch of a box IS the contiguous 64-float window at
    8-float-unit address  img*2073600 + y0*1920 + x0  -- affine in the
    window origin, no per-box block/parity math, f32-exact (< 2^24).
  * One indirect gather per 128-box group (4 total), each [128 part, 64
    floats] with a [128,1] offset AP -- the exact shape proven on HW.
  * Weighted mean = sum(patch * (wx (x) wy)) / 49 with 1/49 folded into
    the weight outer product; fused multiply+accumulate per group.

Sharding (8 cores): boxes are sorted by batch id and split into 8 equal
chunks of 512 (G=4 groups of 128, no padding waste).  Each core receives
the IMG_SLOTS=4 consecutive depth images its chunk can reference
(batch-id-aware routing).  Host unpermutes per-core results.  Boxes whose
chunk would span >4 images or that touch the clip region fall back to an
exact host computation (never happens for the reference distribution).
"""

import os
import sys

import numpy as np

if "/opt/trn_rl_repo" not in sys.path:
    sys.path.insert(0, "/opt/trn_rl_repo")

import concourse.bacc as bacc
import concourse.bass as bass
import concourse.mybir as mybir
import concourse.tile as tile
from concourse.bass_utils import run_bass_kernel_spmd

H, W = 1080, 1920
HW = H * W
B = 16
N_CORES = 8
S = 512          # boxes per core (exact split)
G = S // 128     # 4 free-dim groups of 128 boxes
IMG_SLOTS = 4    # consecutive depth images staged per core
NB8 = HW         # 8-float band units per image (1080*1920 window origins)
F32 = mybir.dt.float32
I32 = mybir.dt.int32
ALU = mybir.AluOpType
AX = mybir.AxisListType

SX3 = 3.0 * (W - 1) / W   # 3 sample steps in pixels, x
SY3 = 3.0 * (H - 1) / H

# box columns in the blob (host pre-scaled so ph = colA + colB per axis):
# [img, x1*959.5 - SX3, y1*539.5 - SY3, x2*959.5, y2*539.5]
BOX_C = 5
# const layout (one row, replicated to 128 partitions):
# [0:2]   clampMax = [W-8, H-8]
# [2:4]   delta    = [SX3-3, SY3-3]          (f = fp' + delta)
# [4:4+16G]  wtc[g][a][j] = (j+1) + 3 - S3[a]  (wt ramp, tiled per group
#            so the wt build op stays 3-D: [128, G*2, 8])
_C_CLMP, _C_DELTA, _C_WTC = 0, 2, 4
_C_TOT = _C_WTC + 16 * G
BLOB_W = G * BOX_C + _C_TOT


def _const_row() -> np.ndarray:
    f = np.float32
    iota = np.arange(1, 9, dtype=np.float64)
    wtc = np.concatenate([(iota + 3.0 - SX3), (iota + 3.0 - SY3)])
    row = np.concatenate([
        np.array([W - 8, H - 8], f),
        np.array([SX3 - 3.0, SY3 - 3.0], f),
        np.tile(wtc, G).astype(f),
    ])
    assert row.shape[0] == _C_TOT
    return np.tile(row[None, :], (128, 1)).astype(f)


def build_nc(n_iters: int = 1, hw_loop: bool = False, unroll: int = 8,
             bufs: int = 2) -> bass.Bass:
    nc = bacc.Bacc()
    # [p, g*5+c] = boxes[g*128+p, c]; [p, G*5:] = per-partition const row
    blob = nc.dram_tensor("blob", [128, BLOB_W], F32, kind="ExternalInput")
    # 8-row bands, column-major: depth[img*HW + y0*W + x0, r] = img[y0+r][x0]
    depth = nc.dram_tensor("depth", [IMG_SLOTS * NB8, 8], F32, kind="ExternalInput")
    avg_out = nc.dram_tensor("avg", [128, G], F32, kind="ExternalOutput")

    with tile.TileContext(nc) as tc:
        with tc.tile_pool(name="p", bufs=(bufs if n_iters > 1 else 1)) as pool:
          def body():
            v = nc.vector
            blob_sb = pool.tile([128, BLOB_W], F32, tag="blob")
            nc.sync.dma_start(out=blob_sb[:], in_=blob[:, :])
            bb = blob_sb[:, 0:G * BOX_C].rearrange("p (g c) -> p g c", g=G)
            cst = blob_sb[:, G * BOX_C:BLOB_W]

            # ---- gather-index chain first (packed x/y in last dim) ----
            ph = pool.tile([128, G, 2], F32, tag="ph")     # first-sample px coords
            v.tensor_tensor(out=ph[:], in0=bb[:, :, 1:3], in1=bb[:, :, 3:5], op=ALU.add)
            # floor(ph), correct whether the cast truncates (CoreSim) or
            # rounds to nearest (silicon): floor = cast_back - (cast_back > ph)
            ri = pool.tile([128, G, 2], I32, tag="ri")
            v.tensor_copy(out=ri[:], in_=ph[:])
            rf = pool.tile([128, G, 2], F32, tag="rf")
            v.tensor_copy(out=rf[:], in_=ri[:])
            mg = pool.tile([128, G, 2], F32, tag="mg")
            v.tensor_tensor(out=mg[:], in0=rf[:], in1=ph[:], op=ALU.is_gt)
            fl = pool.tile([128, G, 2], F32, tag="fl")
            v.tensor_tensor(out=fl[:], in0=rf[:], in1=mg[:], op=ALU.subtract)
            rc = pool.tile([128, G, 2], F32, tag="rc")
            v.tensor_tensor(out=rc[:], in0=fl[:],
                            in1=cst[:, _C_CLMP:_C_CLMP + 2].unsqueeze(1).to_broadcast([128, G, 2]),
                            op=ALU.min)
            xy0 = pool.tile([128, G, 2], F32, tag="xy0")   # (x0, y0) window origin
            v.tensor_scalar(out=xy0[:], in0=rc[:], scalar1=0.0, scalar2=None, op0=ALU.max)

            r0 = pool.tile([128, G, 1], F32, tag="r0")     # y0*W + x0
            v.scalar_tensor_tensor(out=r0[:], in0=xy0[:, :, 1:2], scalar=float(W),
                                   in1=xy0[:, :, 0:1], op0=ALU.mult, op1=ALU.add)
            a0 = pool.tile([128, G, 1], F32, tag="a0")     # img*HW + y0*W + x0
            v.scalar_tensor_tensor(out=a0[:], in0=bb[:, :, 0:1], scalar=float(NB8),
                                   in1=r0[:], op0=ALU.mult, op1=ALU.add)
            idx = pool.tile([128, G, 1], I32, tag="idx")
            v.tensor_copy(out=idx[:], in_=a0[:])

            # ---- one gather per group: 64-float window == the 8x8 patch ----
            st = pool.tile([128, G, 64], F32, tag="st")
            for gi in range(G):
                nc.gpsimd.indirect_dma_start(
                    out=st[:, gi, :],
                    out_offset=None,
                    in_=depth[:, :],
                    in_offset=bass.IndirectOffsetOnAxis(ap=idx[:, gi, :], axis=0),
                )

            # ---- closed-form separable weights (run under the gathers) ----
            fp = pool.tile([128, G, 2], F32, tag="fp")     # ph - xy0 = f - delta
            v.tensor_tensor(out=fp[:], in0=ph[:], in1=xy0[:], op=ALU.subtract)
            wt = pool.tile([128, G * 2, 8], F32, tag="wt")  # [(g a), j]
            v.tensor_tensor(out=wt[:],
                            in0=cst[:, _C_WTC:_C_WTC + 16 * G]
                                .rearrange("p (ga j) -> p ga j", ga=G * 2),
                            in1=fp[:].rearrange("p g a -> p (g a)").unsqueeze(2)
                                .to_broadcast([128, G * 2, 8]),
                            op=ALU.subtract)
            v.tensor_scalar(out=wt[:], in0=wt[:], scalar1=1.0, scalar2=0.0,
                            op0=ALU.min, op1=ALU.max)
            f2 = pool.tile([128, G, 2], F32, tag="f2")     # true f = fp + delta
            v.tensor_tensor(out=f2[:], in0=fp[:],
                            in1=cst[:, _C_DELTA:_C_DELTA + 2].unsqueeze(1).to_broadcast([128, G, 2]),
                            op=ALU.add)
            v.tensor_scalar(out=wt[:, :, 7:8],
                            in0=f2[:].rearrange("p g a -> p (g a)").unsqueeze(2),
                            scalar1=0.0, scalar2=None, op0=ALU.max)
            # weight outer product in band order: wprod[c*8+r] = wx[c]*wy[r]/49
            wprod = pool.tile([128, G, 64], F32, tag="wprod")
            for gi in range(G):
                v.scalar_tensor_tensor(
                    out=wprod[:, gi, :].rearrange("p (c r) -> p c r", c=8),
                    in0=wt[:, 2 * gi, :].unsqueeze(2).to_broadcast([128, 8, 8]),
                    scalar=float(1.0 / 49.0),
                    in1=wt[:, 2 * gi + 1, :].unsqueeze(1).to_broadcast([128, 8, 8]),
                    op0=ALU.mult, op1=ALU.mult)

            # ---- fused multiply + accumulate per group ----
            sm = pool.tile([128, G], F32, tag="sm")
            scratch = pool.tile([128, G, 64], F32, tag="scratch")
            for gi in range(G):
                v.scalar_tensor_tensor(out=scratch[:, gi, :], in0=st[:, gi, :],
                                       scalar=1.0, in1=wprod[:, gi, :],
                                       op0=ALU.mult, op1=ALU.mult,
                                       accum_out=sm[:, gi:gi + 1])
            nc.sync.dma_start(out=avg_out[:, :], in_=sm[:])

          if hw_loop and n_iters > 1:
              assert n_iters % unroll == 0
              with tc.For_i(0, n_iters // unroll):
                  for _u in range(unroll):
                      body()
          else:
              for _it in range(n_iters):
                  body()
    nc.finalize()
    return nc


_NC_CACHE = None


def _get_nc() -> bass.Bass:
    global _NC_CACHE
    if _NC_CACHE is None:
        _NC_CACHE = build_nc()
    return _NC_CACHE


def _host_bilinear(bb: np.ndarray, dm: np.ndarray) -> np.ndarray:
    """Reference-exact fallback for overflow boxes (host, numpy)."""
    f = np.float32
    bids = bb[:, 0].astype(np.int32)
    cx = bb[:, 3] + bb[:, 5] - f(1.0)
    cy = bb[:, 4] + bb[:, 6] - f(1.0)
    offx = np.linspace(-3.0, 3.0, 7).astype(f) / f(W * 0.5)
    offy = np.linspace(-3.0, 3.0, 7).astype(f) / f(H * 0.5)
    gx = np.clip(cx[:, None] + offx[None, :], -1.0, 1.0).astype(f)
    gy = np.clip(cy[:, None] + offy[None, :], -1.0, 1.0).astype(f)
    ix = ((gx + f(1.0)) * f(0.5) * f(W - 1)).astype(f)
    iy = ((gy + f(1.0)) * f(0.5) * f(H - 1)).astype(f)
    x0 = np.floor(ix); y0 = np.floor(iy)
    wx = ix - x0; wy = iy - y0
    x0i = np.clip(x0.astype(np.int32), 0, W - 1); x1i = np.clip(x0i + 1, 0, W - 1)
    y0i = np.clip(y0.astype(np.int32), 0, H - 1); y1i = np.clip(y0i + 1, 0, H - 1)
    d = dm[:, 0]
    bI = bids[:, None, None]
    vv = (d[bI, y0i[:, :, None], x0i[:, None, :]] * (1 - wx)[:, None, :] * (1 - wy)[:, :, None]
          + d[bI, y0i[:, :, None], x1i[:, None, :]] * wx[:, None, :] * (1 - wy)[:, :, None]
          + d[bI, y1i[:, :, None], x0i[:, None, :]] * (1 - wx)[:, None, :] * wy[:, :, None]
          + d[bI, y1i[:, :, None], x1i[:, None, :]] * wx[:, None, :] * wy[:, :, None])
    return vv.mean(axis=(1, 2)).astype(f)


def _make_bands(img: np.ndarray) -> np.ndarray:
    """[H, W] -> [H*W, 8] band layout: out[y*W + x, r] = img[min(y+r, H-1), x].

    Rows below the image edge replicate the last row; they are only ever
    multiplied by exactly-zero weights (y0 <= H-8 after clamping)."""
    padded = np.concatenate([img, np.repeat(img[-1:, :], 7, axis=0)], axis=0)
    v = np.lib.stride_tricks.sliding_window_view(padded, 8, axis=0)  # [H, W, 8]
    return np.ascontiguousarray(v[:H].reshape(H * W, 8))


def make_in_maps(bb: np.ndarray, dm: np.ndarray):
    """Stage per-core inputs.  Returns (in_maps, sels, fallback_idx)."""
    bids = bb[:, 0].astype(np.int32)
    order = np.argsort(bids, kind="stable")
    n = bb.shape[0]
    assert n == N_CORES * S, f"expected {N_CORES * S} boxes, got {n}"
    # boxes in the clip region are approximated wrongly -> host fallback
    cxy = bb[:, 3:5] + bb[:, 5:7] - 1.0
    eps = np.float32(3.2 / (H * 0.5))
    clipbad = (np.abs(cxy) > 1.0 - eps).any(axis=1)
    bands = {}
    in_maps, sels, fallback = [], [], []
    crow = _const_row()
    for c in range(N_CORES):
        sel = order[c * S:(c + 1) * S]
        sels.append(sel)
        bsel = bids[sel]
        lo = int(min(bsel.min(), B - IMG_SLOTS))
        img = bsel - lo
        bad = (img < 0) | (img >= IMG_SLOTS) | clipbad[sel]
        if bad.any():
            fallback.append(sel[bad])
        bsub = bb[sel].astype(np.float64)
        boxes_c = np.empty((S, BOX_C), np.float32)
        boxes_c[:, 0] = np.clip(img, 0, IMG_SLOTS - 1).astype(np.float32)
        boxes_c[:, 1] = (bsub[:, 3] * 959.5 - SX3).astype(np.float32)
        boxes_c[:, 2] = (bsub[:, 4] * 539.5 - SY3).astype(np.float32)
        boxes_c[:, 3] = (bsub[:, 5] * 959.5).astype(np.float32)
        boxes_c[:, 4] = (bsub[:, 6] * 539.5).astype(np.float32)
        blob = np.concatenate(
            [boxes_c.reshape(G, 128, BOX_C).transpose(1, 0, 2).reshape(128, G * BOX_C),
             crow], axis=1).astype(np.float32)
        for i in range(lo, lo + IMG_SLOTS):
            if i not in bands:
                bands[i] = _make_bands(dm[i, 0])
        depth_c = np.concatenate([bands[i] for i in range(lo, lo + IMG_SLOTS)], axis=0)
        in_maps.append({"blob": blob, "depth": depth_c})
    fb = np.concatenate(fallback) if fallback else np.empty(0, np.int64)
    return in_maps, sels, fb


def run(inputs: dict, trace: bool = False):
    """Returns (full_output [N,8] f32, BassKernelResults)."""
    bb = np.ascontiguousarray(np.asarray(inputs["bboxes"], dtype=np.float32))
    dm = np.ascontiguousarray(np.asarray(inputs["depth_map"], dtype=np.float32))
    n = bb.shape[0]
    in_maps, sels, fb = make_in_maps(bb, dm)

    nc = _get_nc()
    if os.environ.get("BASS_KERNEL_SIM") == "1":
        from concourse.bass_interp import CoreSim
        res, br = [], None
        for c in range(N_CORES):
            sim = CoreSim(nc)
            for k_, v_ in in_maps[c].items():
                sim.tensor(k_)[:] = v_
            sim.simulate()
            res.append({"avg": np.array(sim.tensor("avg"))})
    else:
        br = run_bass_kernel_spmd(nc, in_maps, list(range(N_CORES)), trace=trace)
        res = br.results

    avg = np.empty((n, 1), np.float32)
    for c in range(N_CORES):
        # device layout is [p, g]; host order within the chunk is g*128+p
        avg[sels[c], 0] = res[c]["avg"].reshape(128, G).T.reshape(-1)
    if len(fb):
        avg[fb, 0] = _host_bilinear(bb[fb], dm)
    return np.concatenate([bb, avg], axis=1), br


def kernel(**inputs) -> np.ndarray:
    out, _ = run(inputs)
    return out
